# revision 20
# baseline (speedup 1.0000x reference)
"""Trainium2 Bass kernel for bidirectional masked-LSTM + attention pooling + FC head.

Problem (hardcoded shapes): B=64, T=512, E=256, H=512, OH=1024.
  - x [B,T,E] f32, lengths [B] i32, attn_w [T] f32
  - per-direction LSTM weights Wih [4H,E], Whh [4H,H], biases [4H]
  - fc1 [OH,2H]+[OH], fc2 [T,OH]+[T]
  - out: logits [B,T] f32, padded positions = -1e30

Sharding: 8 cores = 4 batch groups (16 seqs) x 2 directions. Each core runs one
direction's full 512-step recurrence for its 16 sequences. Attention pooling is
folded into the recurrence as a masked weighted accumulate (per-(t,b) scale
table precomputed on host, which also implements sequence reversal masking for
the backward direction). The FC head runs on every core; forward/backward
pooled partials are combined with a pairwise AllReduce.

v2 design notes (vs the xp-through-DRAM baseline):
  - The input projection xp = x @ Wih.T + bias is computed INLINE, one 32-step
    block ahead of the recurrence, into a double-buffered SBUF tile. Its
    matmuls are interleaved one-per-step after each step's gate matmuls so
    they fill the PE idle tail while the elementwise chain runs. This removes
    the 32MB/core xp HBM round trip (4096 small DMAs serializing the old run).
  - Whh is fp8 e4m3: LDWEIGHTS bandwidth is the PE floor (64 128x128 tiles
    per step); fp8 FWL halves the weight-load time vs f16. Weights and the
    whole xp path are pre-scaled by 2^9 on host so no fp8 value lands in the
    subnormal range; the scale is undone for free via the ACT engine's affine
    (scale=2^-9) at the gate activations. h stays f16 (mixed-dtype matmul).
  - The o-gate's xp is injected into PSUM by an identity matmul so the
    post-recurrence tail is ACT(sigmoid, psum src) -> DVE(h mul) only.

Layouts (per core):
  h "hidden-tiled" [128, K_CH*16]: h[b, hid] at partition hid%128, col (hid//128)*16+b.
  gates PSUM tiled [128, m*16+b] per gate-chunk m (gate g=m*128+p), gate order
  permuted to [i, f, o, g] so i,f share one sigmoid and g is one tanh.
  xpb SBUF [128, m, t*16+b] per 32-step block.
"""

import os

import numpy as np
import ml_dtypes

import concourse.bass as bass
import concourse.tile as tile
from concourse import bacc, mybir
from concourse.bass_utils import run_bass_kernel_spmd

B, T, E, H, OH = 64, 512, 256, 512, 1024
G = 4 * H          # 2048 gates
BL = 16            # batch per core
M_CH = G // 128    # 16 gate chunks
K_CH = H // 128    # 4 hidden chunks
E_CH = E // 128    # 2 input chunks
MO_CH = OH // 128  # 8
MT_CH = T // 128   # 4
NBLK = 32          # xp block (timesteps)
NB = T // NBLK     # 16 blocks
WSCALE = 512.0     # 2^9 pre-scale on Whh/Wih/bias; undone via ACT scale
INV_WSCALE = 1.0 / WSCALE

f32 = mybir.dt.float32
f16 = mybir.dt.float16
f8 = mybir.dt.float8e4
AF = mybir.ActivationFunctionType
ALU = mybir.AluOpType

# gate permutation: torch order [i,f,g,o] -> kernel order [i,f,o,g]
# perm[new_pos] = old_index  (applied to rows of Wih/Whh and bias)
_GPERM = np.concatenate([
    np.arange(0, H),          # i
    np.arange(H, 2 * H),      # f
    np.arange(3 * H, 4 * H),  # o
    np.arange(2 * H, 3 * H),  # g
])


def _bc_free(ap, reps, width):
    """AP that broadcasts a [P, width] slice to [P, reps, width] via stride-0."""
    return bass.AP(
        tensor=ap.tensor,
        offset=ap.offset,
        ap=[ap.ap[0], [0, reps]] + list(ap.ap[1:]),
    )


def build_nc(n_steps=T, use_collective=True):
    nb = (n_steps + NBLK - 1) // NBLK
    nc = bacc.Bacc("TRN2", target_bir_lowering=False, num_devices=8)

    # ---- DRAM parameters (per-core payloads prepared on host) ----
    xt = nc.declare_dram_parameter("xt", [E_CH, 128, NB, NBLK * BL], f16,
                                   isOutput=False)
    wih = nc.declare_dram_parameter("wih", [E_CH, 128, G], f16, isOutput=False)
    whh = nc.declare_dram_parameter("whh", [K_CH, 128, G], f8, isOutput=False)
    biasT = nc.declare_dram_parameter("biasT", [128, M_CH], f32, isOutput=False)
    sc = nc.declare_dram_parameter("sc", [128, T, BL], f16, isOutput=False)
    ident = nc.declare_dram_parameter("ident", [128, 128], f16, isOutput=False)
    w1t = nc.declare_dram_parameter("w1t", [K_CH, 128, OH], f16, isOutput=False)
    b1T = nc.declare_dram_parameter("b1T", [128, MO_CH], f32, isOutput=False)
    w2t = nc.declare_dram_parameter("w2t", [MO_CH, 128, T], f16, isOutput=False)
    b2T = nc.declare_dram_parameter("b2T", [128, MT_CH], f32, isOutput=False)

    out_logits = nc.declare_dram_parameter("out_logits", [128, MT_CH * BL], f32,
                                           isOutput=True)
    out_pooled = nc.declare_dram_parameter("out_pooled", [128, K_CH * BL], f32,
                                           isOutput=True)

    ar_in = nc.dram_tensor("ar_in", [128, MO_CH * BL], f32)
    ar_out = nc.dram_tensor("ar_out", [128, MO_CH * BL], f32)

    with tile.TileContext(nc) as tc:
        with tc.tile_pool(name="const", bufs=1) as const_pool, \
             tc.tile_pool(name="state", bufs=1) as state_pool:
            whh_sb = const_pool.tile([128, K_CH, G], f8)
            for k in range(K_CH):
                nc.sync.dma_start(out=whh_sb[:, k, :], in_=whh[k])
            wih_sb = const_pool.tile([128, E_CH, G], f16)
            for k in range(E_CH):
                nc.sync.dma_start(out=wih_sb[:, k, :], in_=wih[k])
            biasT_sb = const_pool.tile([128, M_CH], f32)
            nc.sync.dma_start(out=biasT_sb, in_=biasT[:, :])
            sc_sb = const_pool.tile([128, T, BL], f16)
            nc.sync.dma_start(out=sc_sb, in_=sc[:, :, :])
            ident_sb = const_pool.tile([128, 128], f16)
            nc.sync.dma_start(out=ident_sb, in_=ident[:, :])
            w1t_sb = const_pool.tile([128, K_CH, OH], f16)
            for k in range(K_CH):
                nc.sync.dma_start(out=w1t_sb[:, k, :], in_=w1t[k])
            b1T_sb = const_pool.tile([128, MO_CH], f32)
            nc.sync.dma_start(out=b1T_sb, in_=b1T[:, :])
            w2t_sb = const_pool.tile([128, MO_CH, T], f16)
            for k in range(MO_CH):
                nc.sync.dma_start(out=w2t_sb[:, k, :], in_=w2t[k])
            b2T_sb = const_pool.tile([128, MT_CH], f32)
            nc.sync.dma_start(out=b2T_sb, in_=b2T[:, :])

            h_sb = state_pool.tile([128, K_CH * BL], f16)
            c_sb = state_pool.tile([128, 64], f16)
            acc = state_pool.tile([128, K_CH * BL], f32)
            nc.vector.memset(h_sb, 0.0)
            nc.vector.memset(c_sb, 0.0)
            nc.gpsimd.memset(acc, 0.0)

            with tc.tile_pool(name="xtp", bufs=2) as xt_pool, \
                 tc.tile_pool(name="xpb", bufs=2) as xpb_pool, \
                 tc.tile_pool(name="p1ps", bufs=2, space="PSUM") as p1ps, \
                 tc.tile_pool(name="rec_ps", bufs=2, space="PSUM") as rec_ps, \
                 tc.tile_pool(name="work", bufs=2) as work:

                def load_xt(blk):
                    t_ = xt_pool.tile([128, E_CH, NBLK * BL], f16, tag="xt")
                    for e in range(E_CH):
                        nc.sync.dma_start(out=t_[:, e, :], in_=xt[e, :, blk, :])
                    return t_

                def p1_mms(m, xt_t):
                    ps = p1ps.tile([128, 512], f32, tag="p1")
                    for e in range(E_CH):
                        nc.tensor.matmul(
                            ps,
                            lhsT=wih_sb[:, e, m * 128:(m + 1) * 128],
                            rhs=xt_t[:, e, :],
                            start=(e == 0),
                            stop=(e == E_CH - 1),
                        )
                    return ps

                def p1_evict(m, ps, xpb_t, on_act):
                    dst = xpb_t[:, m, :]
                    if on_act:
                        nc.scalar.activation(
                            out=dst, in_=ps, func=AF.Identity,
                            bias=biasT_sb[:, m:m + 1], scale=1.0,
                        )
                    else:
                        nc.vector.tensor_scalar(
                            out=dst, in0=ps,
                            scalar1=biasT_sb[:, m:m + 1], scalar2=None,
                            op0=ALU.add,
                        )

                def inject(ps, lo, hi, ts, xpb_t):
                    # FIRST matmul of the group: start=True sets has_written
                    # for all its columns so the Whh matmuls accumulate onto
                    # xp. (start=True clears the whole BANK's bits, so any
                    # later start in the same bank would corrupt other chunks.)
                    nc.tensor.matmul(
                        ps[:, 0:(hi - lo) * 16],
                        lhsT=ident_sb,
                        rhs=xpb_t[:, lo:hi, ts],
                        start=True, stop=False,
                        skip_group_check=True,
                    )

                def emit_step(t, tloc, xpb_t):
                    ts = slice(tloc * BL, (tloc + 1) * BL)
                    # All three xp injects first: they only depend on xpb, so
                    # the PE executes them while waiting for h(t-1) to land.
                    pif = rec_ps.tile([128, 512], f32, tag="pif")
                    psg = rec_ps.tile([128, 512], f32, tag="psg")
                    pso = rec_ps.tile([128, 512], f32, tag="pso")
                    inject(pif, 0, 8, ts, xpb_t)
                    inject(psg, 12, 16, ts, xpb_t)
                    inject(pso, 8, 12, ts, xpb_t)

                    # Whh bursts: within each gate group, k-chunks 0,1 first so
                    # the burst can begin as soon as h[:, 0:32] of the previous
                    # step lands (h is produced in halves below).
                    def whh_burst(ps, m_lo, m_hi, last):
                        n_j = m_hi - m_lo
                        for kh in range(2):
                            for j, m in enumerate(range(m_lo, m_hi)):
                                for k in (2 * kh, 2 * kh + 1):
                                    nc.tensor.matmul(
                                        ps[:, j * 16:(j + 1) * 16],
                                        lhsT=whh_sb[:, k, m * 128:(m + 1) * 128],
                                        rhs=h_sb[:, k * BL:(k + 1) * BL],
                                        start=False,
                                        stop=(last and kh == 1
                                              and j == n_j - 1 and k == 2 * kh + 1),
                                        skip_group_check=True,
                                    )

                    whh_burst(pif, 0, 8, True)
                    sif = work.tile([128, 128], f16, tag="sif")
                    nc.scalar.activation(out=sif, in_=pif[:, 0:128],
                                         func=AF.Sigmoid, scale=INV_WSCALE)
                    # f*c early, off the critical path (halves)
                    t2 = work.tile([128, 64], f16, tag="t2")
                    nc.vector.tensor_mul(out=t2[:, 0:32], in0=sif[:, 64:96],
                                         in1=c_sb[:, 0:32])
                    nc.vector.tensor_mul(out=t2[:, 32:64], in0=sif[:, 96:128],
                                         in1=c_sb[:, 32:64])

                    whh_burst(psg, 12, 16, True)
                    tg = work.tile([128, 64], f16, tag="tg")
                    nc.scalar.activation(out=tg, in_=psg[:, 0:64],
                                         func=AF.Tanh, scale=INV_WSCALE)

                    whh_burst(pso, 8, 12, True)

                    # cell state + tail, pipelined in k-halves so h[:, 0:32]
                    # unblocks the next step's first Whh pass early.
                    so = work.tile([128, 64], f16, tag="so")
                    nc.scalar.activation(out=so, in_=pso[:, 0:64],
                                         func=AF.Sigmoid, scale=INV_WSCALE)
                    t1 = work.tile([128, 64], f16, tag="t1")
                    tch = work.tile([128, 64], f16, tag="tch")
                    halves = [slice(0, 32), slice(32, 64)]
                    for cs in halves:
                        nc.vector.tensor_mul(out=t1[:, cs], in0=sif[:, cs],
                                             in1=tg[:, cs])
                        nc.vector.tensor_add(out=c_sb[:, cs], in0=t1[:, cs],
                                             in1=t2[:, cs])
                    for cs in halves:
                        nc.scalar.activation(out=tch[:, cs], in_=c_sb[:, cs],
                                             func=AF.Tanh)
                    for cs in halves:
                        nc.vector.tensor_mul(out=h_sb[:, cs], in0=so[:, cs],
                                             in1=tch[:, cs])

                    # attention pooling accumulate
                    pt = work.tile([128, 64], f32, tag="pt")
                    nc.gpsimd.tensor_mul(
                        out=pt, in0=h_sb,
                        in1=_bc_free(sc_sb[:, t, :], K_CH, BL),
                    )
                    nc.gpsimd.tensor_add(out=acc, in0=acc, in1=pt)

                # ---- lead-in: x block 0+1, xp block 0 ----
                xt_cur = load_xt(0)
                xpb_cur = xpb_pool.tile([128, M_CH, NBLK * BL], f16, tag="xpb")
                for m in range(M_CH):
                    ps = p1_mms(m, xt_cur)
                    p1_evict(m, ps, xpb_cur, on_act=(m % 2 == 1))
                xt_next = load_xt(1) if nb > 1 else None

                # ---- main loop: recurrence block k + inline phase-1 k+1 ----
                for blk in range(nb):
                    xpb_next = None
                    if blk + 1 < nb:
                        xpb_next = xpb_pool.tile([128, M_CH, NBLK * BL], f16,
                                                 tag="xpb")
                    xt_fut = load_xt(blk + 2) if blk + 2 < nb else None
                    ps_pend = None
                    for tt in range(min(NBLK, n_steps - blk * NBLK)):
                        emit_step(blk * NBLK + tt, tt, xpb_cur)
                        if xpb_next is not None:
                            if tt % 2 == 0:
                                ps_pend = p1_mms(tt // 2, xt_next)
                            else:
                                p1_evict(tt // 2, ps_pend, xpb_next,
                                         on_act=(tt % 4 == 1))
                    xpb_cur = xpb_next
                    xt_cur, xt_next = xt_next, xt_fut

            # ---- head (state pool still open: reads acc) ----
            with tc.tile_pool(name="head", bufs=1) as head, \
                 tc.tile_pool(name="head_ps", bufs=1, space="PSUM") as head_ps:
                nc.sync.dma_start(out=out_pooled[:, :], in_=acc)
                acch = head.tile([128, K_CH * BL], f16)
                nc.vector.tensor_copy(out=acch, in_=acc)
                ps1 = head_ps.tile([128, MO_CH * BL], f32)
                for mo in range(MO_CH):
                    for k in range(K_CH):
                        nc.tensor.matmul(
                            ps1[:, mo * BL:(mo + 1) * BL],
                            lhsT=w1t_sb[:, k, mo * 128:(mo + 1) * 128],
                            rhs=acch[:, k * BL:(k + 1) * BL],
                            start=(k == 0), stop=(k == K_CH - 1),
                        )
                p1_sb = head.tile([128, MO_CH * BL], f32)
                nc.vector.tensor_copy(out=p1_sb, in_=ps1)
                if use_collective:
                    nc.sync.dma_start(out=ar_in[:, :], in_=p1_sb)
                    nc.gpsimd.collective_compute(
                        "AllReduce",
                        ALU.add,
                        replica_groups=[[0, 1], [2, 3], [4, 5], [6, 7]],
                        ins=[ar_in[:, :].opt()],
                        outs=[ar_out[:, :].opt()],
                    )
                    r_sb = head.tile([128, MO_CH * BL], f32)
                    nc.sync.dma_start(out=r_sb, in_=ar_out[:, :])
                else:
                    r_sb = p1_sb
                h1 = head.tile([128, MO_CH * BL], f16)
                for mo in range(MO_CH):
                    nc.scalar.activation(
                        out=h1[:, mo * BL:(mo + 1) * BL],
                        in_=r_sb[:, mo * BL:(mo + 1) * BL],
                        func=AF.Relu,
                        bias=b1T_sb[:, mo:mo + 1],
                    )
                ps2 = head_ps.tile([128, MT_CH * BL], f32)
                for mt in range(MT_CH):
                    for ko in range(MO_CH):
                        nc.tensor.matmul(
                            ps2[:, mt * BL:(mt + 1) * BL],
                            lhsT=w2t_sb[:, ko, mt * 128:(mt + 1) * 128],
                            rhs=h1[:, ko * BL:(ko + 1) * BL],
                            start=(ko == 0), stop=(ko == MO_CH - 1),
                        )
                lg_sb = head.tile([128, MT_CH * BL], f32)
                for mt in range(MT_CH):
                    nc.vector.tensor_scalar(
                        out=lg_sb[:, mt * BL:(mt + 1) * BL],
                        in0=ps2[:, mt * BL:(mt + 1) * BL],
                        scalar1=b2T_sb[:, mt:mt + 1], scalar2=None,
                        op0=ALU.add,
                    )
                nc.sync.dma_start(out=out_logits[:, :], in_=lg_sb)

    nc.compile()
    return nc


def _tile_kxg(w, n_k, dtype=np.float16):
    """[G, K] weight (already permuted rows) -> [n_k, 128, G] with
    out[k, kk, g] = w[g, k*128+kk]."""
    wt = w.T.astype(np.float32)  # [K, G]
    return np.ascontiguousarray(wt.reshape(n_k, 128, -1)).astype(dtype)


def prep_core_inputs(x_dir, wih_p, whh_p, bias_p, sc_tb, fc1_w, fc1_b,
                     fc2_w, fc2_b, direction):
    """Build the per-core input map. x_dir [BL, T, E] f32 (already reversed for
    bwd), weights already gate-permuted."""
    ins = {}
    # xt [E_CH, 128, NB, NBLK*BL]: xt[e][kk][blk][t*BL+b] = x_dir[b,blk*32+t,e*128+kk]
    xr = x_dir.reshape(BL, NB, NBLK, E)
    xtt = xr.transpose(3, 1, 2, 0).reshape(E_CH, 128, NB, NBLK * BL)
    ins["xt"] = np.ascontiguousarray(xtt).astype(np.float16)
    ins["wih"] = _tile_kxg(wih_p * WSCALE, E_CH)
    ins["whh"] = _tile_kxg(whh_p * WSCALE, K_CH, ml_dtypes.float8_e4m3fn)
    ins["biasT"] = np.ascontiguousarray(
        (bias_p * WSCALE).reshape(M_CH, 128).T).astype(np.float32)
    # sc [128, T, BL] replicated over partitions
    ins["sc"] = np.broadcast_to(
        sc_tb.astype(np.float16)[None, :, :], (128, T, BL)).copy()
    ins["ident"] = np.eye(128, dtype=np.float16)
    w1d = fc1_w[:, direction * H:(direction + 1) * H]  # [OH, H]
    ins["w1t"] = _tile_kxg(w1d, K_CH)
    ins["b1T"] = np.ascontiguousarray(
        fc1_b.reshape(MO_CH, 128).T).astype(np.float32)
    ins["w2t"] = _tile_kxg(fc2_w, MO_CH)
    ins["b2T"] = np.ascontiguousarray(
        fc2_b.reshape(MT_CH, 128).T).astype(np.float32)
    return ins


_NC_CACHE = {}
LAST_RESULT = None


def kernel(x, lengths, attn_w, Wih_f, Whh_f, bih_f, bhh_f,
           Wih_b, Whh_b, bih_b, bhh_b, fc1_w, fc1_b, fc2_w, fc2_b):
    x = np.asarray(x, np.float32)
    lengths = np.asarray(lengths, np.int32)
    attn_w = np.asarray(attn_w, np.float32)
    use_collective = os.environ.get("LSTM_NO_COLLECTIVE", "0") != "1"

    n_steps = int(lengths.max())
    key = (n_steps, use_collective)
    if key not in _NC_CACHE:
        _NC_CACHE[key] = build_nc(n_steps, use_collective)
    nc = _NC_CACHE[key]

    # softmax over attn_w (host glue, exact fp32 as in reference)
    aw = attn_w - attn_w.max()
    e = np.exp(aw)
    scores = (e / e.sum()).astype(np.float32)  # [T]

    tr = np.arange(T)
    # forward sc: sc_f[t, b] = scores[t] * (t < len_b)
    # backward sc: sc_b[tau, b] = scores[len_b-1-tau] * (tau < len_b)
    in_maps = []
    for g in range(4):
        bsl = slice(g * BL, (g + 1) * BL)
        xg = x[bsl]                      # [BL, T, E]
        lg = lengths[bsl]                # [BL]
        mask = tr[:, None] < lg[None, :]  # [T, BL]
        sc_f = scores[:, None] * mask
        idx = np.clip(lg[None, :] - 1 - tr[:, None], 0, T - 1)  # [T, BL]
        sc_b = scores[idx] * mask
        # x reversed per sequence (zeros past length)
        idxc = np.clip(lg[:, None] - 1 - tr[None, :], 0, T - 1)  # [BL, T]
        xrev = np.take_along_axis(xg, idxc[:, :, None], axis=1)
        xrev = xrev * mask.T[:, :, None]

        bias_f = (bih_f + bhh_f)[_GPERM].astype(np.float32)
        bias_b = (bih_b + bhh_b)[_GPERM].astype(np.float32)
        in_maps.append(prep_core_inputs(
            xg, Wih_f[_GPERM], Whh_f[_GPERM], bias_f, sc_f,
            fc1_w, fc1_b, fc2_w, fc2_b, 0))
        in_maps.append(prep_core_inputs(
            xrev, Wih_b[_GPERM], Whh_b[_GPERM], bias_b, sc_b,
            fc1_w, fc1_b, fc2_w, fc2_b, 1))

    trace = os.environ.get("LSTM_TRACE", "0") == "1"
    res = run_bass_kernel_spmd(nc, in_maps, list(range(8)), trace=trace)
    results = res.results
    global LAST_RESULT
    LAST_RESULT = res

    out = np.empty((B, T), np.float32)
    for g in range(4):
        if use_collective:
            lt = results[2 * g]["out_logits"]  # [128, MT_CH*BL]
            lg_out = lt.reshape(128, MT_CH, BL).transpose(2, 1, 0).reshape(BL, T)
        else:
            # host head from pooled partials
            pf = results[2 * g]["out_pooled"]
            pb = results[2 * g + 1]["out_pooled"]
            pooled = np.concatenate(
                [pf.reshape(128, K_CH, BL).transpose(2, 1, 0).reshape(BL, H),
                 pb.reshape(128, K_CH, BL).transpose(2, 1, 0).reshape(BL, H)],
                axis=1)
            h1 = np.maximum(pooled @ fc1_w.T + fc1_b, 0.0)
            lg_out = h1 @ fc2_w.T + fc2_b
        out[g * BL:(g + 1) * BL] = lg_out
    tmask = tr[None, :] < lengths[:, None]
    return np.where(tmask, out, np.float32(-1e30)).astype(np.float32)


# revision 21
# speedup vs baseline: 1.0033x; 1.0033x over previous
"""Trainium2 Bass kernel for bidirectional masked-LSTM + attention pooling + FC head.

Problem (hardcoded shapes): B=64, T=512, E=256, H=512, OH=1024.
  - x [B,T,E] f32, lengths [B] i32, attn_w [T] f32
  - per-direction LSTM weights Wih [4H,E], Whh [4H,H], biases [4H]
  - fc1 [OH,2H]+[OH], fc2 [T,OH]+[T]
  - out: logits [B,T] f32, padded positions = -1e30

Sharding: 8 cores = 4 batch groups (16 seqs) x 2 directions. Each core runs one
direction's full 512-step recurrence for its 16 sequences. Attention pooling is
folded into the recurrence as a masked weighted accumulate (per-(t,b) scale
table precomputed on host, which also implements sequence reversal masking for
the backward direction). The FC head runs on every core; forward/backward
pooled partials are combined with a pairwise AllReduce.

v2 design notes (vs the xp-through-DRAM baseline):
  - The input projection xp = x @ Wih.T + bias is computed INLINE, one 32-step
    block ahead of the recurrence, into a double-buffered SBUF tile. Its
    matmuls are interleaved one-per-step after each step's gate matmuls so
    they fill the PE idle tail while the elementwise chain runs. This removes
    the 32MB/core xp HBM round trip (4096 small DMAs serializing the old run).
  - Whh is fp8 e4m3: LDWEIGHTS bandwidth is the PE floor (64 128x128 tiles
    per step); fp8 FWL halves the weight-load time vs f16. Weights and the
    whole xp path are pre-scaled by 2^9 on host so no fp8 value lands in the
    subnormal range; the scale is undone for free via the ACT engine's affine
    (scale=2^-9) at the gate activations. h stays f16 (mixed-dtype matmul).
  - The o-gate's xp is injected into PSUM by an identity matmul so the
    post-recurrence tail is ACT(sigmoid, psum src) -> DVE(h mul) only.

Layouts (per core):
  h "hidden-tiled" [128, K_CH*16]: h[b, hid] at partition hid%128, col (hid//128)*16+b.
  gates PSUM tiled [128, m*16+b] per gate-chunk m (gate g=m*128+p), gate order
  permuted to [i, f, o, g] so i,f share one sigmoid and g is one tanh.
  xpb SBUF [128, m, t*16+b] per 32-step block.
"""

import os

import numpy as np
import ml_dtypes

import concourse.bass as bass
import concourse.tile as tile
from concourse import bacc, mybir
from concourse.bass_utils import run_bass_kernel_spmd

B, T, E, H, OH = 64, 512, 256, 512, 1024
G = 4 * H          # 2048 gates
BL = 16            # batch per core
M_CH = G // 128    # 16 gate chunks
K_CH = H // 128    # 4 hidden chunks
E_CH = E // 128    # 2 input chunks
MO_CH = OH // 128  # 8
MT_CH = T // 128   # 4
NBLK = 32          # xp block (timesteps)
NB = T // NBLK     # 16 blocks
WSCALE = 512.0     # 2^9 pre-scale on Whh/Wih/bias; undone via ACT scale
INV_WSCALE = 1.0 / WSCALE

f32 = mybir.dt.float32
f16 = mybir.dt.float16
f8 = mybir.dt.float8e4
AF = mybir.ActivationFunctionType
ALU = mybir.AluOpType

# gate permutation: torch order [i,f,g,o] -> kernel order [i,f,o,g]
# perm[new_pos] = old_index  (applied to rows of Wih/Whh and bias)
_GPERM = np.concatenate([
    np.arange(0, H),          # i
    np.arange(H, 2 * H),      # f
    np.arange(3 * H, 4 * H),  # o
    np.arange(2 * H, 3 * H),  # g
])


def _bc_free(ap, reps, width):
    """AP that broadcasts a [P, width] slice to [P, reps, width] via stride-0."""
    return bass.AP(
        tensor=ap.tensor,
        offset=ap.offset,
        ap=[ap.ap[0], [0, reps]] + list(ap.ap[1:]),
    )


def build_nc(n_steps=T, use_collective=True):
    nb = (n_steps + NBLK - 1) // NBLK
    nc = bacc.Bacc("TRN2", target_bir_lowering=False, num_devices=8)

    # ---- DRAM parameters (per-core payloads prepared on host) ----
    xt = nc.declare_dram_parameter("xt", [E_CH, 128, NB, NBLK * BL], f16,
                                   isOutput=False)
    wih = nc.declare_dram_parameter("wih", [E_CH, 128, G], f16, isOutput=False)
    whh = nc.declare_dram_parameter("whh", [K_CH, 128, G], f8, isOutput=False)
    biasT = nc.declare_dram_parameter("biasT", [128, M_CH], f32, isOutput=False)
    sc = nc.declare_dram_parameter("sc", [128, T, BL], f16, isOutput=False)
    ident = nc.declare_dram_parameter("ident", [128, 128], f16, isOutput=False)
    w1t = nc.declare_dram_parameter("w1t", [K_CH, 128, OH], f16, isOutput=False)
    b1T = nc.declare_dram_parameter("b1T", [128, MO_CH], f32, isOutput=False)
    w2t = nc.declare_dram_parameter("w2t", [MO_CH, 128, T], f16, isOutput=False)
    b2T = nc.declare_dram_parameter("b2T", [128, MT_CH], f32, isOutput=False)

    out_logits = nc.declare_dram_parameter("out_logits", [128, MT_CH * BL], f32,
                                           isOutput=True)
    out_pooled = nc.declare_dram_parameter("out_pooled", [128, K_CH * BL], f32,
                                           isOutput=True)

    ar_in = nc.dram_tensor("ar_in", [128, MO_CH * BL], f32)
    ar_out = nc.dram_tensor("ar_out", [128, MO_CH * BL], f32)

    with tile.TileContext(nc) as tc:
        with tc.tile_pool(name="const", bufs=1) as const_pool, \
             tc.tile_pool(name="state", bufs=1) as state_pool:
            whh_sb = const_pool.tile([128, K_CH, G], f8)
            for k in range(K_CH):
                nc.sync.dma_start(out=whh_sb[:, k, :], in_=whh[k])
            wih_sb = const_pool.tile([128, E_CH, G], f16)
            for k in range(E_CH):
                nc.sync.dma_start(out=wih_sb[:, k, :], in_=wih[k])
            biasT_sb = const_pool.tile([128, M_CH], f32)
            nc.sync.dma_start(out=biasT_sb, in_=biasT[:, :])
            sc_sb = const_pool.tile([128, T, BL], f16)
            nc.sync.dma_start(out=sc_sb, in_=sc[:, :, :])
            ident_sb = const_pool.tile([128, 128], f16)
            nc.sync.dma_start(out=ident_sb, in_=ident[:, :])
            w1t_sb = const_pool.tile([128, K_CH, OH], f16)
            for k in range(K_CH):
                nc.sync.dma_start(out=w1t_sb[:, k, :], in_=w1t[k])
            b1T_sb = const_pool.tile([128, MO_CH], f32)
            nc.sync.dma_start(out=b1T_sb, in_=b1T[:, :])
            w2t_sb = const_pool.tile([128, MO_CH, T], f16)
            for k in range(MO_CH):
                nc.sync.dma_start(out=w2t_sb[:, k, :], in_=w2t[k])
            b2T_sb = const_pool.tile([128, MT_CH], f32)
            nc.sync.dma_start(out=b2T_sb, in_=b2T[:, :])

            h_sb = state_pool.tile([128, K_CH * BL], f16)
            c_sb = state_pool.tile([128, 64], f16)
            acc = state_pool.tile([128, K_CH * BL], f32)
            nc.vector.memset(h_sb, 0.0)
            nc.vector.memset(c_sb, 0.0)
            nc.gpsimd.memset(acc, 0.0)

            with tc.tile_pool(name="xtp", bufs=2) as xt_pool, \
                 tc.tile_pool(name="xpb", bufs=2) as xpb_pool, \
                 tc.tile_pool(name="p1ps", bufs=2, space="PSUM") as p1ps, \
                 tc.tile_pool(name="rec_ps", bufs=2, space="PSUM") as rec_ps, \
                 tc.tile_pool(name="work", bufs=2) as work:

                def load_xt(blk):
                    t_ = xt_pool.tile([128, E_CH, NBLK * BL], f16, tag="xt")
                    for e in range(E_CH):
                        nc.sync.dma_start(out=t_[:, e, :], in_=xt[e, :, blk, :])
                    return t_

                def p1_mms(m, xt_t):
                    ps = p1ps.tile([128, 512], f32, tag="p1")
                    for e in range(E_CH):
                        nc.tensor.matmul(
                            ps,
                            lhsT=wih_sb[:, e, m * 128:(m + 1) * 128],
                            rhs=xt_t[:, e, :],
                            start=(e == 0),
                            stop=(e == E_CH - 1),
                        )
                    return ps

                def p1_evict(m, ps, xpb_t, on_act):
                    dst = xpb_t[:, m, :]
                    if on_act:
                        nc.scalar.activation(
                            out=dst, in_=ps, func=AF.Identity,
                            bias=biasT_sb[:, m:m + 1], scale=1.0,
                        )
                    else:
                        nc.vector.tensor_scalar(
                            out=dst, in0=ps,
                            scalar1=biasT_sb[:, m:m + 1], scalar2=None,
                            op0=ALU.add,
                        )

                def inject(ps, lo, hi, ts, xpb_t):
                    # FIRST matmul of the group: start=True sets has_written
                    # for all its columns so the Whh matmuls accumulate onto
                    # xp. (start=True clears the whole BANK's bits, so any
                    # later start in the same bank would corrupt other chunks.)
                    nc.tensor.matmul(
                        ps[:, 0:(hi - lo) * 16],
                        lhsT=ident_sb,
                        rhs=xpb_t[:, lo:hi, ts],
                        start=True, stop=False,
                        skip_group_check=True,
                    )

                def emit_step(t, tloc, xpb_t):
                    ts = slice(tloc * BL, (tloc + 1) * BL)
                    # All three xp injects first: they only depend on xpb, so
                    # the PE executes them while waiting for h(t-1) to land.
                    pif = rec_ps.tile([128, 512], f32, tag="pif")
                    psg = rec_ps.tile([128, 512], f32, tag="psg")
                    pso = rec_ps.tile([128, 512], f32, tag="pso")
                    inject(pif, 0, 8, ts, xpb_t)
                    inject(psg, 12, 16, ts, xpb_t)
                    inject(pso, 8, 12, ts, xpb_t)

                    def whh_part(ps, m_lo, m_hi, col0=0, stop=False):
                        n_j = m_hi - m_lo
                        for j, m in enumerate(range(m_lo, m_hi)):
                            for k in range(K_CH):
                                nc.tensor.matmul(
                                    ps[:, col0 + j * 16:col0 + (j + 1) * 16],
                                    lhsT=whh_sb[:, k, m * 128:(m + 1) * 128],
                                    rhs=h_sb[:, k * BL:(k + 1) * BL],
                                    start=False,
                                    stop=(stop and j == n_j - 1
                                          and k == K_CH - 1),
                                    skip_group_check=True,
                                )

                    # i-gates then f-gates; split sigmoids so sigma(i)'s PSUM
                    # drain latency hides under the f-burst.
                    whh_part(pif, 0, 4)
                    whh_part(pif, 4, 8, col0=64, stop=True)
                    si = work.tile([128, 64], f16, tag="si")
                    nc.scalar.activation(out=si, in_=pif[:, 0:64],
                                         func=AF.Sigmoid, scale=INV_WSCALE)
                    sf = work.tile([128, 64], f16, tag="sf")
                    nc.scalar.activation(out=sf, in_=pif[:, 64:128],
                                         func=AF.Sigmoid, scale=INV_WSCALE)
                    # f*c early, off the critical path
                    t2 = work.tile([128, 64], f16, tag="t2")
                    nc.vector.tensor_mul(out=t2, in0=sf, in1=c_sb)

                    whh_part(psg, 12, 16, stop=True)
                    tg = work.tile([128, 64], f16, tag="tg")
                    nc.scalar.activation(out=tg, in_=psg[:, 0:64],
                                         func=AF.Tanh, scale=INV_WSCALE)

                    whh_part(pso, 8, 12, stop=True)

                    # cell state: c = i*g + (f*c from above)
                    t1 = work.tile([128, 64], f16, tag="t1")
                    nc.vector.tensor_mul(out=t1, in0=si, in1=tg)
                    nc.vector.tensor_add(out=c_sb, in0=t1, in1=t2)

                    # tail: sigmoid(o) off PSUM, tanh(c), then h
                    so = work.tile([128, 64], f16, tag="so")
                    nc.scalar.activation(out=so, in_=pso[:, 0:64],
                                         func=AF.Sigmoid, scale=INV_WSCALE)
                    tch = work.tile([128, 64], f16, tag="tch")
                    nc.scalar.activation(out=tch, in_=c_sb, func=AF.Tanh)
                    nc.vector.tensor_mul(out=h_sb, in0=so, in1=tch)

                    # attention pooling accumulate
                    pt = work.tile([128, 64], f32, tag="pt")
                    nc.gpsimd.tensor_mul(
                        out=pt, in0=h_sb,
                        in1=_bc_free(sc_sb[:, t, :], K_CH, BL),
                    )
                    nc.gpsimd.tensor_add(out=acc, in0=acc, in1=pt)

                # ---- lead-in: x block 0+1, xp block 0 ----
                xt_cur = load_xt(0)
                xpb_cur = xpb_pool.tile([128, M_CH, NBLK * BL], f16, tag="xpb")
                for m in range(M_CH):
                    ps = p1_mms(m, xt_cur)
                    p1_evict(m, ps, xpb_cur, on_act=(m % 2 == 1))
                xt_next = load_xt(1) if nb > 1 else None

                # ---- main loop: recurrence block k + inline phase-1 k+1 ----
                for blk in range(nb):
                    xpb_next = None
                    if blk + 1 < nb:
                        xpb_next = xpb_pool.tile([128, M_CH, NBLK * BL], f16,
                                                 tag="xpb")
                    xt_fut = load_xt(blk + 2) if blk + 2 < nb else None
                    ps_pend = None
                    for tt in range(min(NBLK, n_steps - blk * NBLK)):
                        emit_step(blk * NBLK + tt, tt, xpb_cur)
                        if xpb_next is not None:
                            if tt % 2 == 0:
                                ps_pend = p1_mms(tt // 2, xt_next)
                            else:
                                p1_evict(tt // 2, ps_pend, xpb_next,
                                         on_act=(tt % 4 == 1))
                    xpb_cur = xpb_next
                    xt_cur, xt_next = xt_next, xt_fut

            # ---- head (state pool still open: reads acc) ----
            with tc.tile_pool(name="head", bufs=1) as head, \
                 tc.tile_pool(name="head_ps", bufs=1, space="PSUM") as head_ps:
                nc.sync.dma_start(out=out_pooled[:, :], in_=acc)
                acch = head.tile([128, K_CH * BL], f16)
                nc.vector.tensor_copy(out=acch, in_=acc)
                ps1 = head_ps.tile([128, MO_CH * BL], f32)
                for mo in range(MO_CH):
                    for k in range(K_CH):
                        nc.tensor.matmul(
                            ps1[:, mo * BL:(mo + 1) * BL],
                            lhsT=w1t_sb[:, k, mo * 128:(mo + 1) * 128],
                            rhs=acch[:, k * BL:(k + 1) * BL],
                            start=(k == 0), stop=(k == K_CH - 1),
                        )
                p1_sb = head.tile([128, MO_CH * BL], f32)
                nc.vector.tensor_copy(out=p1_sb, in_=ps1)
                if use_collective:
                    nc.sync.dma_start(out=ar_in[:, :], in_=p1_sb)
                    nc.gpsimd.collective_compute(
                        "AllReduce",
                        ALU.add,
                        replica_groups=[[0, 1], [2, 3], [4, 5], [6, 7]],
                        ins=[ar_in[:, :].opt()],
                        outs=[ar_out[:, :].opt()],
                    )
                    r_sb = head.tile([128, MO_CH * BL], f32)
                    nc.sync.dma_start(out=r_sb, in_=ar_out[:, :])
                else:
                    r_sb = p1_sb
                h1 = head.tile([128, MO_CH * BL], f16)
                for mo in range(MO_CH):
                    nc.scalar.activation(
                        out=h1[:, mo * BL:(mo + 1) * BL],
                        in_=r_sb[:, mo * BL:(mo + 1) * BL],
                        func=AF.Relu,
                        bias=b1T_sb[:, mo:mo + 1],
                    )
                ps2 = head_ps.tile([128, MT_CH * BL], f32)
                for mt in range(MT_CH):
                    for ko in range(MO_CH):
                        nc.tensor.matmul(
                            ps2[:, mt * BL:(mt + 1) * BL],
                            lhsT=w2t_sb[:, ko, mt * 128:(mt + 1) * 128],
                            rhs=h1[:, ko * BL:(ko + 1) * BL],
                            start=(ko == 0), stop=(ko == MO_CH - 1),
                        )
                lg_sb = head.tile([128, MT_CH * BL], f32)
                for mt in range(MT_CH):
                    nc.vector.tensor_scalar(
                        out=lg_sb[:, mt * BL:(mt + 1) * BL],
                        in0=ps2[:, mt * BL:(mt + 1) * BL],
                        scalar1=b2T_sb[:, mt:mt + 1], scalar2=None,
                        op0=ALU.add,
                    )
                nc.sync.dma_start(out=out_logits[:, :], in_=lg_sb)

    nc.compile()
    return nc


def _tile_kxg(w, n_k, dtype=np.float16):
    """[G, K] weight (already permuted rows) -> [n_k, 128, G] with
    out[k, kk, g] = w[g, k*128+kk]."""
    wt = w.T.astype(np.float32)  # [K, G]
    return np.ascontiguousarray(wt.reshape(n_k, 128, -1)).astype(dtype)


def prep_core_inputs(x_dir, wih_p, whh_p, bias_p, sc_tb, fc1_w, fc1_b,
                     fc2_w, fc2_b, direction):
    """Build the per-core input map. x_dir [BL, T, E] f32 (already reversed for
    bwd), weights already gate-permuted."""
    ins = {}
    # xt [E_CH, 128, NB, NBLK*BL]: xt[e][kk][blk][t*BL+b] = x_dir[b,blk*32+t,e*128+kk]
    xr = x_dir.reshape(BL, NB, NBLK, E)
    xtt = xr.transpose(3, 1, 2, 0).reshape(E_CH, 128, NB, NBLK * BL)
    ins["xt"] = np.ascontiguousarray(xtt).astype(np.float16)
    ins["wih"] = _tile_kxg(wih_p * WSCALE, E_CH)
    ins["whh"] = _tile_kxg(whh_p * WSCALE, K_CH, ml_dtypes.float8_e4m3fn)
    ins["biasT"] = np.ascontiguousarray(
        (bias_p * WSCALE).reshape(M_CH, 128).T).astype(np.float32)
    # sc [128, T, BL] replicated over partitions
    ins["sc"] = np.broadcast_to(
        sc_tb.astype(np.float16)[None, :, :], (128, T, BL)).copy()
    ins["ident"] = np.eye(128, dtype=np.float16)
    w1d = fc1_w[:, direction * H:(direction + 1) * H]  # [OH, H]
    ins["w1t"] = _tile_kxg(w1d, K_CH)
    ins["b1T"] = np.ascontiguousarray(
        fc1_b.reshape(MO_CH, 128).T).astype(np.float32)
    ins["w2t"] = _tile_kxg(fc2_w, MO_CH)
    ins["b2T"] = np.ascontiguousarray(
        fc2_b.reshape(MT_CH, 128).T).astype(np.float32)
    return ins


_NC_CACHE = {}
LAST_RESULT = None


def kernel(x, lengths, attn_w, Wih_f, Whh_f, bih_f, bhh_f,
           Wih_b, Whh_b, bih_b, bhh_b, fc1_w, fc1_b, fc2_w, fc2_b):
    x = np.asarray(x, np.float32)
    lengths = np.asarray(lengths, np.int32)
    attn_w = np.asarray(attn_w, np.float32)
    use_collective = os.environ.get("LSTM_NO_COLLECTIVE", "0") != "1"

    n_steps = int(lengths.max())
    key = (n_steps, use_collective)
    if key not in _NC_CACHE:
        _NC_CACHE[key] = build_nc(n_steps, use_collective)
    nc = _NC_CACHE[key]

    # softmax over attn_w (host glue, exact fp32 as in reference)
    aw = attn_w - attn_w.max()
    e = np.exp(aw)
    scores = (e / e.sum()).astype(np.float32)  # [T]

    tr = np.arange(T)
    # forward sc: sc_f[t, b] = scores[t] * (t < len_b)
    # backward sc: sc_b[tau, b] = scores[len_b-1-tau] * (tau < len_b)
    in_maps = []
    for g in range(4):
        bsl = slice(g * BL, (g + 1) * BL)
        xg = x[bsl]                      # [BL, T, E]
        lg = lengths[bsl]                # [BL]
        mask = tr[:, None] < lg[None, :]  # [T, BL]
        sc_f = scores[:, None] * mask
        idx = np.clip(lg[None, :] - 1 - tr[:, None], 0, T - 1)  # [T, BL]
        sc_b = scores[idx] * mask
        # x reversed per sequence (zeros past length)
        idxc = np.clip(lg[:, None] - 1 - tr[None, :], 0, T - 1)  # [BL, T]
        xrev = np.take_along_axis(xg, idxc[:, :, None], axis=1)
        xrev = xrev * mask.T[:, :, None]

        bias_f = (bih_f + bhh_f)[_GPERM].astype(np.float32)
        bias_b = (bih_b + bhh_b)[_GPERM].astype(np.float32)
        in_maps.append(prep_core_inputs(
            xg, Wih_f[_GPERM], Whh_f[_GPERM], bias_f, sc_f,
            fc1_w, fc1_b, fc2_w, fc2_b, 0))
        in_maps.append(prep_core_inputs(
            xrev, Wih_b[_GPERM], Whh_b[_GPERM], bias_b, sc_b,
            fc1_w, fc1_b, fc2_w, fc2_b, 1))

    trace = os.environ.get("LSTM_TRACE", "0") == "1"
    res = run_bass_kernel_spmd(nc, in_maps, list(range(8)), trace=trace)
    results = res.results
    global LAST_RESULT
    LAST_RESULT = res

    out = np.empty((B, T), np.float32)
    for g in range(4):
        if use_collective:
            lt = results[2 * g]["out_logits"]  # [128, MT_CH*BL]
            lg_out = lt.reshape(128, MT_CH, BL).transpose(2, 1, 0).reshape(BL, T)
        else:
            # host head from pooled partials
            pf = results[2 * g]["out_pooled"]
            pb = results[2 * g + 1]["out_pooled"]
            pooled = np.concatenate(
                [pf.reshape(128, K_CH, BL).transpose(2, 1, 0).reshape(BL, H),
                 pb.reshape(128, K_CH, BL).transpose(2, 1, 0).reshape(BL, H)],
                axis=1)
            h1 = np.maximum(pooled @ fc1_w.T + fc1_b, 0.0)
            lg_out = h1 @ fc2_w.T + fc2_b
        out[g * BL:(g + 1) * BL] = lg_out
    tmask = tr[None, :] < lengths[:, None]
    return np.where(tmask, out, np.float32(-1e30)).astype(np.float32)
